# revision 26
# baseline (speedup 1.0000x reference)
"""MultiHeadedAttention Trainium2 Bass kernel.

Reference (per batch element b, full shapes B=8, S=1024, D=512, H=8, DK=64):
    Q = x_q @ Wq + bq ; K = x_k @ Wk + bk ; V = x_v @ Wv + bv   (per-head split)
    S = Q K^T / sqrt(DK);  S masked where mask==0 -> -inf
    P = softmax(S); P zeroed where mask==0
    Y = (P V, heads concat) @ Wo + bo

Sharding: pure data parallel over batch — core c computes batch element c.
No collectives.

Mask compaction: the mask zeroes whole key positions (same for every query
and head of a batch element), and masked columns contribute exactly 0 to
both the PV accumulation and the softmax denominator. The host gathers only
the unmasked key/value positions, pads to a multiple of 128 with bias
-30000 slots (exp == 0), and the kernel loops over KT = ceil(Lmax/128)
k-tiles instead of 8 (SPMD: all cores run the max tile count across the
batch). Worst case (no masking) equals the dense kernel. The tile count is
locked on first use of get_nc/make_in_maps so cached modules and input maps
always agree.

Per-core layout (f32r matmuls for scores/out; bf16 projections and PV;
PSUM accumulates f32):
  xT        [in=512, S|SK] host-transposed bf16 (halves input HBM traffic);
                         q-path split across both HWDGE queues, k-path on
                         the Pool SWDGE queue, ACT issues only the smalls
  QT        [feat, S]    psum[out128, q512] += Wq.T @ xT; bias via DVE
                         tensor_scalar_add (per-partition scalar);
                         only out-tile o=0 runs before attention, o=1..3
                         stream as PE filler inside pairs 0..2 (1-buf tag)
  KT        [feat, SK]   same, over compacted positions
  V natural [SK, feat]   psum += xT_v.T @ Wv; bv folded into the DVE PSUM
                         drain via a Pool-broadcast bias plane; stored bf16
                         interleaved as v_aug[row128, head, 65] with a ones
                         column per head (softmax denominator for free);
                         V chains are pair-0 fillers
  S^T       [k128, q512] = KT_h[d64, k128].T @ QT_h[d64, q512]
                         head pairs packed into PE row groups 0/64 via
                         tile_position -> both matmuls run concurrently
  P^T       = Exp(S^T/8 + maskbias_k)  (ACT, [128,1024] per sub, bf16 out)
  (PV)^T+den[65, q512]  += v_aug_h[k128, 65].T @ P^T[k128, q512] (row 64 =
                         denominator); last pair streams BOTH heads so the
                         tail is short
  norm      DVE recip -> Pool partition_broadcast -> DVE mul into
            at_pair[t][h%2*64 :+64, q]
  Y natural [q128, 512] += at_pair[t][:, q128].T @ Wo[feat128, out512];
            bo folded into the DVE drain; yps tiles rotate through the
            ov_ps bufs so chains start while the tail normalizes run.
"""

import numpy as np

B, S, D, H = 8, 1024, 512, 8
DK = D // H  # 64
P = 128
KI = D // P  # 4 in-feature tiles
RT = S // P  # 8 row tiles
QC = S // 512  # 2 q chunks of 512
HP = H // 2  # 4 head pairs
MASK_NEG = -30000.0  # exp(-30000) == 0.0 in f32

_CACHED = {}
_KT = None
_N_BODIES = 1  # analysis knob: pipelined bodies in the non-loop build


def _build_nc(loop_reps=None, kt=RT):
    import concourse.mybir as mybir
    import concourse.tile as tile
    from concourse import bacc

    f32 = mybir.dt.float32
    f32r = mybir.dt.float32r
    bf16 = mybir.dt.bfloat16
    EXP = mybir.ActivationFunctionType.Exp
    ISCALE = 1.0 / float(np.sqrt(DK))
    SK = kt * P

    nc = bacc.Bacc("TRN2")

    xqT_d = nc.dram_tensor("xqT", (KI, P, S), bf16, kind="ExternalInput")
    xkT_d = nc.dram_tensor("xkT", (KI, P, SK), bf16, kind="ExternalInput")
    xvT_d = nc.dram_tensor("xvT", (KI, P, SK), bf16, kind="ExternalInput")
    maskb_d = nc.dram_tensor("maskb", (P, kt), f32, kind="ExternalInput")
    wq_d = nc.dram_tensor("wq", (KI, P, D), bf16, kind="ExternalInput")
    wk_d = nc.dram_tensor("wk", (KI, P, D), bf16, kind="ExternalInput")
    wv_d = nc.dram_tensor("wv", (KI, P, D), bf16, kind="ExternalInput")
    wo_d = nc.dram_tensor("wo", (KI, P, D), bf16, kind="ExternalInput")
    bq_d = nc.dram_tensor("bq", (P, KI), f32, kind="ExternalInput")
    bk_d = nc.dram_tensor("bk", (P, KI), f32, kind="ExternalInput")
    bv_d = nc.dram_tensor("bv", (1, D), bf16, kind="ExternalInput")
    bo_d = nc.dram_tensor("bo", (1, D), f32r, kind="ExternalInput")
    y_d = nc.dram_tensor("y", (RT, P, D), f32, kind="ExternalOutput")

    # projection chain chunking: q is always 2x512; k covers SK
    QCH = [(c0, 512) for c0 in range(0, S, 512)]
    KCH = [(c0, min(512, SK - c0)) for c0 in range(0, SK, 512)]

    with tile.TileContext(nc) as tc, nc.allow_low_precision(
        reason="bf16 inputs/P/V with fp32 PSUM accumulation stays within tolerance"
    ):
        from contextlib import ExitStack

        def body(proj_pool=None, side=None):
            """Three-phase body generator for software pipelining.

            yield #1: head done (input DMA issued, o=0 projections emitted)
            yield #2: pairs+norms done — yields this body's fil pool so the
                      NEXT body's head can borrow it for o=0 projections
            then:     out-projection tail, scopes close.

            In the benchmark loop bodies interleave as
              headA pairsA | headB tailA | pairsB tailB
            so the next body's projections run on PE before the previous
            body's out-projection instead of behind it.
            """
            with ExitStack() as ctx:
                const = ctx.enter_context(tc.tile_pool(name="const", bufs=1, side=side))
                persist = ctx.enter_context(tc.tile_pool(name="persist", bufs=1, side=side))

                wq = [const.tile([P, D], bf16, name=f"wq{i}", tag=f"wq{i}") for i in range(KI)]
                wk = [const.tile([P, D], bf16, name=f"wk{i}", tag=f"wk{i}") for i in range(KI)]
                wv = [const.tile([P, D], bf16, name=f"wv{i}", tag=f"wv{i}") for i in range(KI)]
                wo = [const.tile([P, D], bf16, name=f"wo{i}", tag=f"wo{i}") for i in range(KI)]
                bq_t = const.tile([P, KI], f32, name="bq_t", tag="bq")
                bk_t = const.tile([P, KI], f32, name="bk_t", tag="bk")
                bv_t = const.tile([1, D], bf16, name="bv_t", tag="bv")
                bo_t = const.tile([1, D], f32r, name="bo_t", tag="bo")
                maskb = const.tile([P, kt], f32, name="maskb", tag="maskb")
                bv_full = const.tile([P, D], bf16, name="bv_full", tag="bvf")
                bo_full = const.tile([P, D], f32r, name="bo_full", tag="bof")

                # persistent intermediates
                qt = [persist.tile([P, S], bf16, name=f"qt{i}", tag=f"qt{i}") for i in range(KI)]
                kt_ = [persist.tile([P, SK], bf16, name=f"kt{i}", tag=f"kt{i}") for i in range(KI)]
                v_aug = [persist.tile([P, H, DK + 1], bf16, name=f"va{i}", tag=f"va{i}") for i in range(kt)]
                # head-pair attention outputs: pair t rows 0:64 = head 2t,
                # rows 64:128 = head 2t+1 => feature rows 128t..128t+127
                at = [persist.tile([P, S], bf16, name=f"at{i}", tag=f"at{i}") for i in range(HP)]

                xt_pool = ctx.enter_context(tc.tile_pool(name="xt", bufs=1, side=side))
                xqT = [xt_pool.tile([P, S], bf16, name=f"xq{i}", tag=f"xq{i}") for i in range(KI)]
                xkT = [xt_pool.tile([P, SK], bf16, name=f"xk{i}", tag=f"xk{i}") for i in range(KI)]
                xvT = [xt_pool.tile([P, SK], bf16, name=f"xv{i}", tag=f"xv{i}") for i in range(KI)]

                # --- DMA split by need-time: q-path striped across both
                # HWDGE queues (single HWDGE device serializes copies, so
                # what matters is front-of-queue order); k/v on the Pool
                # SWDGE queue which runs in parallel with HWDGE; smalls
                # behind scalar's q share; wo last.
                for i in range(0, KI, 2):
                    nc.sync.dma_start(wq[i][:], wq_d[i])
                    nc.sync.dma_start(xqT[i][:], xqT_d[i])
                    nc.scalar.dma_start(wq[i + 1][:], wq_d[i + 1])
                    nc.scalar.dma_start(xqT[i + 1][:], xqT_d[i + 1])
                for i in range(KI):
                    nc.gpsimd.dma_start(wk[i][:], wk_d[i])
                    nc.gpsimd.dma_start(xkT[i][:], xkT_d[i])
                nc.scalar.dma_start(maskb[:], maskb_d[:])
                nc.scalar.dma_start(bq_t[:], bq_d[:])
                nc.scalar.dma_start(bk_t[:], bk_d[:])
                nc.scalar.dma_start(bv_t[:], bv_d[:])
                for i in range(0, KI, 2):
                    nc.sync.dma_start(wv[i][:], wv_d[i])
                    nc.sync.dma_start(xvT[i][:], xvT_d[i])
                    nc.gpsimd.dma_start(wv[i + 1][:], wv_d[i + 1])
                    nc.gpsimd.dma_start(xvT[i + 1][:], xvT_d[i + 1])
                for i in range(KI):
                    nc.gpsimd.dma_start(wo[i][:], wo_d[i])
                nc.gpsimd.dma_start(bo_t[:], bo_d[:])
                nc.gpsimd.partition_broadcast(bv_full[:], bv_t[:])
                nc.gpsimd.partition_broadcast(bo_full[:], bo_t[:])

                def qk_chain(pool, tag, which, o, c0, w):
                    """One projection chain: psum = sum_ki W^T x (+bias via DVE)."""
                    wgt, x, bias, dst = which
                    ps = pool.tile([P, 512], f32, name="pps", tag=tag)
                    for ki in range(KI):
                        nc.tensor.matmul(
                            ps[:, 0:w],
                            wgt[ki][:, o * P : (o + 1) * P],
                            x[ki][:, c0 : c0 + w],
                            start=(ki == 0),
                            stop=(ki == KI - 1),
                        )
                    nc.vector.tensor_scalar_add(
                        dst[o][:, c0 : c0 + w],
                        ps[:, 0:w],
                        bias[:, o : o + 1],
                    )

                def v_chain(pool, tag, rt):
                    """V projection row-tile rt -> v_aug[rt] (bf16, +ones col)."""
                    ps = pool.tile([P, 512], f32, name="vps", tag=tag)
                    for ki in range(KI):
                        nc.tensor.matmul(
                            ps[:],
                            xvT[ki][:, rt * P : (rt + 1) * P],
                            wv[ki][:],
                            start=(ki == 0),
                            stop=(ki == KI - 1),
                        )
                    nc.vector.tensor_add(
                        v_aug[rt][:, :, 0:DK],
                        ps[:].rearrange("p (h d) -> p h d", h=H),
                        bv_full[:].rearrange("p (h d) -> p h d", h=H),
                    )
                    nc.vector.memset(v_aug[rt][:, :, DK : DK + 1], 1.0)

                QK = [(wq, xqT, bq_t, qt), (wk, xkT, bk_t, kt_)]
                CHAINS = [(0, c0, w) for c0, w in QCH] + [(1, c0, w) for c0, w in KCH]

                # o=0 projections up front (scores for pair 0 need them);
                # inside the pipeline they borrow the previous body's fil
                # bank instead of opening a 2-bank pool of their own
                if proj_pool is not None:
                    for which, c0, w in CHAINS:
                        qk_chain(proj_pool, "fil", QK[which], 0, c0, w)
                else:
                    with ExitStack() as actx:
                        psA = actx.enter_context(
                            tc.tile_pool(name="psA", bufs=2, space="PSUM", side=side)
                        )
                        for which, c0, w in CHAINS:
                            qk_chain(psA, "psA", QK[which], 0, c0, w)

                yield None  # head done

                # --- attention, one head pair at a time; V-proj and the
                # remaining Q/K projections stream through a 1-buf PSUM tag
                # as PE filler between score/PV matmuls ---
                with ExitStack() as bctx:
                    pt_pool = bctx.enter_context(
                        tc.tile_pool(name="pt", bufs=2 * kt + 2, side=side)
                    )
                    rec_pool = bctx.enter_context(tc.tile_pool(name="rec", bufs=4, side=side))
                    at_ps = bctx.enter_context(
                        tc.tile_pool(name="spsum", bufs=2, space="PSUM", side=side)
                    )
                    ov_ps = bctx.enter_context(
                        tc.tile_pool(name="opsum", bufs=3, space="PSUM", side=side)
                    )
                    fil_ps = bctx.enter_context(
                        tc.tile_pool(name="filpsum", bufs=1, space="PSUM", side=side)
                    )

                    def normalize(t, sub, qc, ops):
                        """at[t][sub] = ops[0:DK] / ops[DK] for one q-chunk."""
                        off = sub * DK
                        rec = rec_pool.tile([1, 512], f32, name="rec", tag="rec")
                        nc.vector.reciprocal(rec[:], ops[DK : DK + 1, :])
                        rbs = rec_pool.tile([DK, 512], f32, name="rbs", tag="rbs")
                        nc.gpsimd.partition_broadcast(rbs[:], rec[:])
                        nc.vector.tensor_mul(
                            at[t][off : off + DK, qc * 512 : (qc + 1) * 512],
                            ops[0:DK, :],
                            rbs[:],
                        )

                    last_norm = []
                    for t in range(HP):
                        pts = [
                            [pt_pool.tile([P, S], bf16, name="pt", tag="pt") for _ in range(kt)]
                            for _ in range(2)
                        ]
                        # sub 0's PV chains consume pt tiles in lockstep with
                        # the exp stream so half the pool frees at pair end
                        # (the next pair's exps then aren't slot-starved).
                        ops00 = ov_ps.tile([P, 512], f32, name="ops", tag="ops")
                        ops01 = ov_ps.tile([P, 512], f32, name="ops", tag="ops")
                        if t == HP - 1:
                            # last pair: stream sub 1 too (filler bank is free
                            # — no more projections) so the tail is short
                            ops10 = fil_ps.tile([P, 512], f32, name="ops", tag="fil")
                            ops11 = ov_ps.tile([P, 512], f32, name="ops", tag="ops")
                        # PE filler thunks for this pair, spread evenly over
                        # the kt iterations (V_j always lands by iteration j)
                        if t == 0:
                            fillers = [
                                (lambda j=j: v_chain(fil_ps, "fil", j))
                                for j in range(kt)
                            ] + [
                                (lambda c=c: qk_chain(fil_ps, "fil", QK[c[0]], 1, c[1], c[2]))
                                for c in CHAINS
                            ]
                        elif t < HP - 1:
                            fillers = [
                                (lambda c=c, o=t + 1: qk_chain(fil_ps, "fil", QK[c[0]], o, c[1], c[2]))
                                for c in CHAINS
                            ]
                        else:
                            fillers = []
                        emitted = 0
                        for ktile in range(kt):
                            for sub in range(2):
                                off = sub * DK
                                sps = at_ps.tile([P, S], f32, name="sps", tag="sps")
                                for qc in range(QC):
                                    nc.tensor.matmul(
                                        sps[:, qc * 512 : (qc + 1) * 512],
                                        kt_[t][off : off + DK, ktile * P : (ktile + 1) * P],
                                        qt[t][off : off + DK, qc * 512 : (qc + 1) * 512],
                                        start=True,
                                        stop=True,
                                        tile_position=(off, 0),
                                    )
                                nc.scalar.activation(
                                    pts[sub][ktile][:],
                                    sps[:],
                                    EXP,
                                    bias=maskb[:, ktile : ktile + 1],
                                    scale=ISCALE,
                                )
                            # fillers behind the scores so they never block
                            # the exp stream
                            want = (ktile + 1) * len(fillers) // kt
                            while emitted < want:
                                fillers[emitted]()
                                emitted += 1
                            for qc, ops in ((0, ops00), (1, ops01)):
                                nc.tensor.matmul(
                                    ops[0 : DK + 1, :],
                                    v_aug[ktile][:, 2 * t, 0 : DK + 1],
                                    pts[0][ktile][:, qc * 512 : (qc + 1) * 512],
                                    start=(ktile == 0),
                                    stop=(ktile == kt - 1),
                                )
                            if t == HP - 1:
                                for qc, ops in ((0, ops10), (1, ops11)):
                                    nc.tensor.matmul(
                                        ops[0 : DK + 1, :],
                                        v_aug[ktile][:, 2 * t + 1, 0 : DK + 1],
                                        pts[1][ktile][:, qc * 512 : (qc + 1) * 512],
                                        start=(ktile == 0),
                                        stop=(ktile == kt - 1),
                                    )
                        if t < HP - 1:
                            normalize(t, 0, 0, ops00)
                            normalize(t, 0, 1, ops01)
                            for qc in range(QC):
                                ops = ov_ps.tile([P, 512], f32, name="ops", tag="ops")
                                for ktile in range(kt):
                                    nc.tensor.matmul(
                                        ops[0 : DK + 1, :],
                                        v_aug[ktile][:, 2 * t + 1, 0 : DK + 1],
                                        pts[1][ktile][:, qc * 512 : (qc + 1) * 512],
                                        start=(ktile == 0),
                                        stop=(ktile == kt - 1),
                                    )
                                normalize(t, 1, qc, ops)
                        else:
                            # qc0 normalizes first so the first half of the
                            # output projection can start; qc1 runs on DVE/
                            # Pool concurrently with those matmuls
                            normalize(t, 0, 0, ops00)
                            normalize(t, 1, 0, ops10)
                            last_norm = [(t, 0, 1, ops01), (t, 1, 1, ops11)]

                    # qc1 normalizes of the last pair: DVE/Pool work that
                    # overlaps the first output-projection matmuls
                    for args in last_norm:
                        normalize(*args)

                    yield fil_ps  # pairs done; next body may borrow fil

                    # --- output projection: contraction K=128 over head
                    # pairs; yps tiles rotate through the ov_ps bufs, whose
                    # banks free earliest (right after the normalizes read
                    # them), so the first chains start during the tail norms
                    y_pool = bctx.enter_context(tc.tile_pool(name="y", bufs=3, side=side))
                    for rt in range(RT):
                        yps = (
                            fil_ps.tile([P, D], f32, name="yps", tag="fil")
                            if rt % 3 == 2
                            else ov_ps.tile([P, D], f32, name="yps", tag="ops")
                        )
                        for t in range(HP):
                            nc.tensor.matmul(
                                yps[:],
                                at[t][:, rt * P : (rt + 1) * P],
                                wo[t][:],
                                start=(t == 0),
                                stop=(t == HP - 1),
                            )
                        yt = y_pool.tile([P, D], f32, name="yt", tag="yt")
                        nc.vector.tensor_add(
                            yt[:], yps[:], bo_full[:].bitcast(f32)
                        )
                        nc.scalar.dma_start(y_d[rt], yt[:])

        def run_bodies(n):
            """Emit n software-pipelined bodies:
            head_i | tail_{i-1} | pairs_i | head_{i+1} | tail_i | ..."""
            prev = None
            prev_fil = None
            for i in range(n):
                g = body(prev_fil, side="right" if i % 2 else "left")
                next(g)  # head
                if prev is not None:
                    for _ in prev:  # tail of previous body, scopes close
                        pass
                prev_fil = next(g)  # pairs + norms
                prev = g
            if prev is not None:
                for _ in prev:
                    pass

        if loop_reps is None:
            run_bodies(_N_BODIES)
        else:
            # benchmark variant: repeat on-device; two pipelined bodies per
            # loop iteration (step=2 keeps the total rep count intact) so
            # consecutive bodies get alternating tile allocations and the
            # next body's projections aren't queued behind the previous
            # body's out-projection.
            ET = mybir.EngineType
            with tc.For_i(
                0,
                loop_reps,
                2,
                hint_engines=(ET.PE, ET.Activation, ET.DVE, ET.SP, ET.Pool),
            ):
                run_bodies(2)

    nc.compile()
    return nc


def get_nc(loop_reps=None):
    global _KT
    if _KT is None:
        _KT = RT  # no inputs seen yet: lock the worst case
    key = ("nc", loop_reps, _KT)
    if key not in _CACHED:
        _CACHED[key] = _build_nc(loop_reps, _KT)
    return _CACHED[key]


def make_in_maps(query, key, value, mask, Wq, bq, Wk, bk, Wv, bv, Wo, bo):
    """Shard full inputs into per-core input maps (host-side numpy)."""
    global _KT
    f = np.float32
    query = np.asarray(query, f)
    key = np.asarray(key, f)
    value = np.asarray(value, f)
    mask = np.asarray(mask)

    import ml_dtypes

    bf = ml_dtypes.bfloat16

    # mask compaction: keep only unmasked key positions, padded to 128
    pos = [np.nonzero(mask[c, 0] != 0)[0] for c in range(B)]
    need = max(1, -(-max(len(p) for p in pos) // P))
    if _KT is None:
        _KT = need
    elif need > _KT:
        _KT = need  # stale cached modules can't serve this input
        _CACHED.clear()
    SK = _KT * P

    def wtiles(W, dt=bf):
        return np.ascontiguousarray(np.asarray(W, f).reshape(KI, P, D).astype(dt))

    wq_t, wk_t, wv_t = wtiles(Wq), wtiles(Wk), wtiles(Wv)
    wo_t = wtiles(Wo)
    bq_t = np.ascontiguousarray(np.asarray(bq, f).reshape(KI, P).T)
    bk_t = np.ascontiguousarray(np.asarray(bk, f).reshape(KI, P).T)
    bv_t = np.ascontiguousarray(np.asarray(bv, f).reshape(1, D).astype(bf))
    bo_t = np.ascontiguousarray(np.asarray(bo, f).reshape(1, D))

    in_maps = []
    for c in range(B):
        L = len(pos[c])
        idx = np.zeros(SK, dtype=np.int64)
        idx[:L] = pos[c]
        mb = np.full(SK, f(MASK_NEG), dtype=f)
        mb[:L] = 0.0
        xqT = np.ascontiguousarray(query[c].T.astype(bf)).reshape(KI, P, S)
        xkT = np.ascontiguousarray(key[c][idx].T.astype(bf)).reshape(KI, P, SK)
        xvT = np.ascontiguousarray(value[c][idx].T.astype(bf)).reshape(KI, P, SK)
        mb = np.ascontiguousarray(mb.reshape(_KT, P).T)
        in_maps.append(
            {
                "xqT": xqT,
                "xkT": xkT,
                "xvT": xvT,
                "maskb": mb,
                "wq": wq_t,
                "wk": wk_t,
                "wv": wv_t,
                "wo": wo_t,
                "bq": bq_t,
                "bk": bk_t,
                "bv": bv_t,
                "bo": bo_t,
            }
        )
    return in_maps


def kernel(**inputs):
    from concourse.bass_utils import run_bass_kernel_spmd

    in_maps = make_in_maps(**inputs)  # before get_nc: locks the k-tile count
    nc = get_nc()
    res = run_bass_kernel_spmd(nc, in_maps, core_ids=list(range(B)))
    out = np.stack([res.results[c]["y"].reshape(S, D) for c in range(B)])
    return out.astype(np.float32)


# revision 29
# speedup vs baseline: 1.0144x; 1.0144x over previous
"""MultiHeadedAttention Trainium2 Bass kernel.

Reference (per batch element b, full shapes B=8, S=1024, D=512, H=8, DK=64):
    Q = x_q @ Wq + bq ; K = x_k @ Wk + bk ; V = x_v @ Wv + bv   (per-head split)
    S = Q K^T / sqrt(DK);  S masked where mask==0 -> -inf
    P = softmax(S); P zeroed where mask==0
    Y = (P V, heads concat) @ Wo + bo

Sharding: pure data parallel over batch — core c computes batch element c.
No collectives.

Mask compaction: the mask zeroes whole key positions (same for every query
and head of a batch element), and masked columns contribute exactly 0 to
both the PV accumulation and the softmax denominator. The host gathers only
the unmasked key/value positions, pads to a multiple of 128 with bias
-30000 slots (exp == 0), and the kernel loops over KT = ceil(Lmax/128)
k-tiles instead of 8 (SPMD: all cores run the max tile count across the
batch). Worst case (no masking) equals the dense kernel. The tile count is
locked on first use of get_nc/make_in_maps so cached modules and input maps
always agree.

Per-core layout (bf16 operands throughout the matmuls — same PE speed as
f32r at these shapes, half the SBUF/HBM; every accumulation is f32 in PSUM
and the softmax normalization runs in f32):
  xT        [in=512, S|SK] host-transposed bf16 (halves input HBM traffic);
                         q-path split across both HWDGE queues, k-path on
                         the Pool SWDGE queue, ACT issues only the smalls
  QT        [feat, S]    psum[out128, q512] += Wq.T @ xT; bias via DVE
                         tensor_scalar_add (per-partition scalar);
                         only out-tile o=0 runs before attention, o=1..3
                         stream as PE filler inside pairs 0..2 (1-buf tag)
  KT        [feat, SK]   same, over compacted positions
  V natural [SK, feat]   psum += xT_v.T @ Wv; bv folded into the DVE PSUM
                         drain via a Pool-broadcast bias plane; stored bf16
                         interleaved as v_aug[row128, head, 65] with a ones
                         column per head (softmax denominator for free);
                         V chains are pair-0 fillers
  S^T       [k128, q512] = KT_h[d64, k128].T @ QT_h[d64, q512]
                         head pairs packed into PE row groups 0/64 via
                         tile_position -> both matmuls run concurrently
  P^T       = Exp(S^T/8 + maskbias_k)  (ACT, [128,1024] per sub, bf16 out)
  (PV)^T+den[65, q512]  += v_aug_h[k128, 65].T @ P^T[k128, q512] (row 64 =
                         denominator); last pair streams BOTH heads so the
                         tail is short
  norm      DVE recip -> Pool partition_broadcast -> DVE mul into
            at_pair[t][h%2*64 :+64, q]
  Y natural [q128, 512] += at_pair[t][:, q128].T @ Wo[feat128, out512];
            bo folded into the DVE drain; yps tiles rotate through the
            ov_ps bufs so chains start while the tail normalizes run.
"""

import numpy as np

B, S, D, H = 8, 1024, 512, 8
DK = D // H  # 64
P = 128
KI = D // P  # 4 in-feature tiles
RT = S // P  # 8 row tiles
QC = S // 512  # 2 q chunks of 512
HP = H // 2  # 4 head pairs
MASK_NEG = -30000.0  # exp(-30000) == 0.0 in f32

_CACHED = {}
_KT = None
_N_BODIES = 1  # analysis knob: pipelined bodies in the non-loop build


def _build_nc(loop_reps=None, kt=RT):
    import concourse.mybir as mybir
    import concourse.tile as tile
    from concourse import bacc

    f32 = mybir.dt.float32
    f32r = mybir.dt.float32r
    bf16 = mybir.dt.bfloat16
    EXP = mybir.ActivationFunctionType.Exp
    ISCALE = 1.0 / float(np.sqrt(DK))
    SK = kt * P

    nc = bacc.Bacc("TRN2")

    xqT_d = nc.dram_tensor("xqT", (KI, P, S), bf16, kind="ExternalInput")
    xkT_d = nc.dram_tensor("xkT", (KI, P, SK), bf16, kind="ExternalInput")
    xvT_d = nc.dram_tensor("xvT", (KI, P, SK), bf16, kind="ExternalInput")
    maskb_d = nc.dram_tensor("maskb", (P, kt), f32, kind="ExternalInput")
    wq_d = nc.dram_tensor("wq", (KI, P, D), bf16, kind="ExternalInput")
    wk_d = nc.dram_tensor("wk", (KI, P, D), bf16, kind="ExternalInput")
    wv_d = nc.dram_tensor("wv", (KI, P, D), bf16, kind="ExternalInput")
    wo_d = nc.dram_tensor("wo", (KI, P, D), bf16, kind="ExternalInput")
    bq_d = nc.dram_tensor("bq", (P, KI), f32, kind="ExternalInput")
    bk_d = nc.dram_tensor("bk", (P, KI), f32, kind="ExternalInput")
    bv_d = nc.dram_tensor("bv", (1, D), bf16, kind="ExternalInput")
    bo_d = nc.dram_tensor("bo", (1, D), f32r, kind="ExternalInput")
    y_d = nc.dram_tensor("y", (RT, P, D), f32, kind="ExternalOutput")

    # projection chain chunking: q is always 2x512; k covers SK
    QCH = [(c0, 512) for c0 in range(0, S, 512)]
    KCH = [(c0, min(512, SK - c0)) for c0 in range(0, SK, 512)]

    with tile.TileContext(nc) as tc, nc.allow_low_precision(
        reason="bf16 inputs/P/V with fp32 PSUM accumulation stays within tolerance"
    ):
        from contextlib import ExitStack

        def body(proj_pool=None, side=None):
            """Three-phase body generator for software pipelining.

            yield #1: head done (input DMA issued, o=0 projections emitted)
            yield #2: pairs+norms done — yields this body's ov pool so the
                      NEXT body's head can borrow it for o=0 projections
            then:     out-projection tail, scopes close.

            In the benchmark loop bodies interleave as
              headA pairsA | headB tailA | pairsB tailB
            so the next body's projections run on PE before the previous
            body's out-projection instead of behind it.
            """
            with ExitStack() as ctx:
                const = ctx.enter_context(tc.tile_pool(name="const", bufs=1, side=side))
                persist = ctx.enter_context(tc.tile_pool(name="persist", bufs=1, side=side))

                wq = [const.tile([P, D], bf16, name=f"wq{i}", tag=f"wq{i}") for i in range(KI)]
                wk = [const.tile([P, D], bf16, name=f"wk{i}", tag=f"wk{i}") for i in range(KI)]
                wv = [const.tile([P, D], bf16, name=f"wv{i}", tag=f"wv{i}") for i in range(KI)]
                wo = [const.tile([P, D], bf16, name=f"wo{i}", tag=f"wo{i}") for i in range(KI)]
                bq_t = const.tile([P, KI], f32, name="bq_t", tag="bq")
                bk_t = const.tile([P, KI], f32, name="bk_t", tag="bk")
                bv_t = const.tile([1, D], bf16, name="bv_t", tag="bv")
                bo_t = const.tile([1, D], f32r, name="bo_t", tag="bo")
                maskb = const.tile([P, kt], f32, name="maskb", tag="maskb")
                bv_full = const.tile([P, D], bf16, name="bv_full", tag="bvf")
                bo_full = const.tile([P, D], f32r, name="bo_full", tag="bof")

                # persistent intermediates
                qt = [persist.tile([P, S], bf16, name=f"qt{i}", tag=f"qt{i}") for i in range(KI)]
                kt_ = [persist.tile([P, SK], bf16, name=f"kt{i}", tag=f"kt{i}") for i in range(KI)]
                v_aug = [persist.tile([P, H, DK + 1], bf16, name=f"va{i}", tag=f"va{i}") for i in range(kt)]
                # head-pair attention outputs: pair t rows 0:64 = head 2t,
                # rows 64:128 = head 2t+1 => feature rows 128t..128t+127
                at = [persist.tile([P, S], bf16, name=f"at{i}", tag=f"at{i}") for i in range(HP)]

                xt_pool = ctx.enter_context(tc.tile_pool(name="xt", bufs=1, side=side))
                xqT = [xt_pool.tile([P, S], bf16, name=f"xq{i}", tag=f"xq{i}") for i in range(KI)]
                xkT = [xt_pool.tile([P, SK], bf16, name=f"xk{i}", tag=f"xk{i}") for i in range(KI)]
                xvT = [xt_pool.tile([P, SK], bf16, name=f"xv{i}", tag=f"xv{i}") for i in range(KI)]

                # --- DMA split by need-time: q-path striped across both
                # HWDGE queues (single HWDGE device serializes copies, so
                # what matters is front-of-queue order); k/v on the Pool
                # SWDGE queue which runs in parallel with HWDGE; smalls
                # behind scalar's q share; wo last.
                for i in range(0, KI, 2):
                    nc.sync.dma_start(wq[i][:], wq_d[i])
                    nc.sync.dma_start(xqT[i][:], xqT_d[i])
                    nc.scalar.dma_start(wq[i + 1][:], wq_d[i + 1])
                    nc.scalar.dma_start(xqT[i + 1][:], xqT_d[i + 1])
                for i in range(KI):
                    nc.gpsimd.dma_start(wk[i][:], wk_d[i])
                    nc.gpsimd.dma_start(xkT[i][:], xkT_d[i])
                nc.scalar.dma_start(maskb[:], maskb_d[:])
                nc.scalar.dma_start(bq_t[:], bq_d[:])
                nc.scalar.dma_start(bk_t[:], bk_d[:])
                nc.scalar.dma_start(bv_t[:], bv_d[:])
                for i in range(0, KI, 2):
                    nc.sync.dma_start(wv[i][:], wv_d[i])
                    nc.sync.dma_start(xvT[i][:], xvT_d[i])
                    nc.gpsimd.dma_start(wv[i + 1][:], wv_d[i + 1])
                    nc.gpsimd.dma_start(xvT[i + 1][:], xvT_d[i + 1])
                for i in range(KI):
                    nc.gpsimd.dma_start(wo[i][:], wo_d[i])
                nc.gpsimd.dma_start(bo_t[:], bo_d[:])
                nc.gpsimd.partition_broadcast(bv_full[:], bv_t[:])
                nc.gpsimd.partition_broadcast(bo_full[:], bo_t[:])

                def qk_chain(pool, tag, which, o, c0, w):
                    """One projection chain: psum = sum_ki W^T x (+bias via DVE)."""
                    wgt, x, bias, dst = which
                    ps = pool.tile([P, 512], f32, name="pps", tag=tag)
                    for ki in range(KI):
                        nc.tensor.matmul(
                            ps[:, 0:w],
                            wgt[ki][:, o * P : (o + 1) * P],
                            x[ki][:, c0 : c0 + w],
                            start=(ki == 0),
                            stop=(ki == KI - 1),
                        )
                    nc.vector.tensor_scalar_add(
                        dst[o][:, c0 : c0 + w],
                        ps[:, 0:w],
                        bias[:, o : o + 1],
                    )

                def v_chain(pool, tag, rt):
                    """V projection row-tile rt -> v_aug[rt] (bf16, +ones col)."""
                    ps = pool.tile([P, 512], f32, name="vps", tag=tag)
                    for ki in range(KI):
                        nc.tensor.matmul(
                            ps[:],
                            xvT[ki][:, rt * P : (rt + 1) * P],
                            wv[ki][:],
                            start=(ki == 0),
                            stop=(ki == KI - 1),
                        )
                    nc.vector.tensor_add(
                        v_aug[rt][:, :, 0:DK],
                        ps[:].rearrange("p (h d) -> p h d", h=H),
                        bv_full[:].rearrange("p (h d) -> p h d", h=H),
                    )
                    nc.vector.memset(v_aug[rt][:, :, DK : DK + 1], 1.0)

                QK = [(wq, xqT, bq_t, qt), (wk, xkT, bk_t, kt_)]
                CHAINS = [(0, c0, w) for c0, w in QCH] + [(1, c0, w) for c0, w in KCH]

                # o=0 projections up front (scores for pair 0 need them);
                # inside the pipeline they borrow the previous body's ov
                # pool (3 bufs -> pipelined chains, no 1-buf stall) instead
                # of opening a 2-bank pool of their own
                if proj_pool is not None:
                    for which, c0, w in CHAINS:
                        qk_chain(proj_pool, "ops", QK[which], 0, c0, w)
                else:
                    with ExitStack() as actx:
                        psA = actx.enter_context(
                            tc.tile_pool(name="psA", bufs=2, space="PSUM", side=side)
                        )
                        for which, c0, w in CHAINS:
                            qk_chain(psA, "psA", QK[which], 0, c0, w)

                yield None  # head done

                # --- attention, one head pair at a time; V-proj and the
                # remaining Q/K projections stream through a 1-buf PSUM tag
                # as PE filler between score/PV matmuls ---
                with ExitStack() as bctx:
                    pt_pool = bctx.enter_context(
                        tc.tile_pool(name="pt", bufs=2 * kt + 4, side=side)
                    )
                    rec_pool = bctx.enter_context(tc.tile_pool(name="rec", bufs=4, side=side))
                    at_ps = bctx.enter_context(
                        tc.tile_pool(name="spsum", bufs=2, space="PSUM", side=side)
                    )
                    ov_ps = bctx.enter_context(
                        tc.tile_pool(name="opsum", bufs=3, space="PSUM", side=side)
                    )
                    fil_ps = bctx.enter_context(
                        tc.tile_pool(name="filpsum", bufs=1, space="PSUM", side=side)
                    )

                    def normalize(t, sub, qc, ops):
                        """at[t][sub] = ops[0:DK] / ops[DK] for one q-chunk."""
                        off = sub * DK
                        rec = rec_pool.tile([1, 512], f32, name="rec", tag="rec")
                        nc.vector.reciprocal(rec[:], ops[DK : DK + 1, :])
                        rbs = rec_pool.tile([DK, 512], f32, name="rbs", tag="rbs")
                        nc.gpsimd.partition_broadcast(rbs[:], rec[:])
                        nc.vector.tensor_mul(
                            at[t][off : off + DK, qc * 512 : (qc + 1) * 512],
                            ops[0:DK, :],
                            rbs[:],
                        )

                    last_norm = []
                    for t in range(HP):
                        pts = [
                            [pt_pool.tile([P, S], bf16, name="pt", tag="pt") for _ in range(kt)]
                            for _ in range(2)
                        ]
                        # sub 0's PV chains consume pt tiles in lockstep with
                        # the exp stream so half the pool frees at pair end
                        # (the next pair's exps then aren't slot-starved).
                        ops00 = ov_ps.tile([P, 512], f32, name="ops", tag="ops")
                        ops01 = ov_ps.tile([P, 512], f32, name="ops", tag="ops")
                        if t == HP - 1:
                            # last pair: stream sub 1 too (filler bank is free
                            # — no more projections) so the tail is short
                            ops10 = fil_ps.tile([P, 512], f32, name="ops", tag="fil")
                            ops11 = ov_ps.tile([P, 512], f32, name="ops", tag="ops")
                        # PE filler thunks for this pair, spread evenly over
                        # the kt iterations (V_j always lands by iteration j)
                        if t == 0:
                            fillers = [
                                (lambda j=j: v_chain(fil_ps, "fil", j))
                                for j in range(kt)
                            ] + [
                                (lambda c=c: qk_chain(fil_ps, "fil", QK[c[0]], 1, c[1], c[2]))
                                for c in CHAINS
                            ]
                        elif t < HP - 1:
                            fillers = [
                                (lambda c=c, o=t + 1: qk_chain(fil_ps, "fil", QK[c[0]], o, c[1], c[2]))
                                for c in CHAINS
                            ]
                        else:
                            fillers = []
                        emitted = 0
                        for ktile in range(kt):
                            for sub in range(2):
                                off = sub * DK
                                sps = at_ps.tile([P, S], f32, name="sps", tag="sps")
                                for qc in range(QC):
                                    nc.tensor.matmul(
                                        sps[:, qc * 512 : (qc + 1) * 512],
                                        kt_[t][off : off + DK, ktile * P : (ktile + 1) * P],
                                        qt[t][off : off + DK, qc * 512 : (qc + 1) * 512],
                                        start=True,
                                        stop=True,
                                        tile_position=(off, 0),
                                    )
                                nc.scalar.activation(
                                    pts[sub][ktile][:],
                                    sps[:],
                                    EXP,
                                    bias=maskb[:, ktile : ktile + 1],
                                    scale=ISCALE,
                                )
                            # fillers behind the scores so they never block
                            # the exp stream
                            want = (ktile + 1) * len(fillers) // kt
                            while emitted < want:
                                fillers[emitted]()
                                emitted += 1
                            for qc, ops in ((0, ops00), (1, ops01)):
                                nc.tensor.matmul(
                                    ops[0 : DK + 1, :],
                                    v_aug[ktile][:, 2 * t, 0 : DK + 1],
                                    pts[0][ktile][:, qc * 512 : (qc + 1) * 512],
                                    start=(ktile == 0),
                                    stop=(ktile == kt - 1),
                                )
                            if t == HP - 1:
                                for qc, ops in ((0, ops10), (1, ops11)):
                                    nc.tensor.matmul(
                                        ops[0 : DK + 1, :],
                                        v_aug[ktile][:, 2 * t + 1, 0 : DK + 1],
                                        pts[1][ktile][:, qc * 512 : (qc + 1) * 512],
                                        start=(ktile == 0),
                                        stop=(ktile == kt - 1),
                                    )
                        if t < HP - 1:
                            normalize(t, 0, 0, ops00)
                            normalize(t, 0, 1, ops01)
                            for qc in range(QC):
                                ops = ov_ps.tile([P, 512], f32, name="ops", tag="ops")
                                for ktile in range(kt):
                                    nc.tensor.matmul(
                                        ops[0 : DK + 1, :],
                                        v_aug[ktile][:, 2 * t + 1, 0 : DK + 1],
                                        pts[1][ktile][:, qc * 512 : (qc + 1) * 512],
                                        start=(ktile == 0),
                                        stop=(ktile == kt - 1),
                                    )
                                normalize(t, 1, qc, ops)
                        else:
                            # qc0 normalizes first so the first half of the
                            # output projection can start; qc1 runs on DVE/
                            # Pool concurrently with those matmuls
                            normalize(t, 0, 0, ops00)
                            normalize(t, 1, 0, ops10)
                            last_norm = [(t, 0, 1, ops01), (t, 1, 1, ops11)]

                    # qc1 normalizes of the last pair: DVE/Pool work that
                    # overlaps the first output-projection matmuls
                    for args in last_norm:
                        normalize(*args)

                    yield ov_ps  # pairs done; next body may borrow it for projections

                    # --- output projection: contraction K=128 over head
                    # pairs; yps tiles rotate through the ov_ps bufs, whose
                    # banks free earliest (right after the normalizes read
                    # them), so the first chains start during the tail norms
                    y_pool = bctx.enter_context(tc.tile_pool(name="y", bufs=3, side=side))
                    for rt in range(RT):
                        yps = (
                            fil_ps.tile([P, D], f32, name="yps", tag="fil")
                            if rt % 3 == 2
                            else ov_ps.tile([P, D], f32, name="yps", tag="ops")
                        )
                        for t in range(HP):
                            nc.tensor.matmul(
                                yps[:],
                                at[t][:, rt * P : (rt + 1) * P],
                                wo[t][:],
                                start=(t == 0),
                                stop=(t == HP - 1),
                            )
                        yt = y_pool.tile([P, D], f32, name="yt", tag="yt")
                        nc.vector.tensor_add(
                            yt[:], yps[:], bo_full[:].bitcast(f32)
                        )
                        nc.scalar.dma_start(y_d[rt], yt[:])

        def run_bodies(n):
            """Emit n software-pipelined bodies:
            head_i | tail_{i-1} | pairs_i | head_{i+1} | tail_i | ..."""
            prev = None
            prev_fil = None
            for i in range(n):
                g = body(prev_fil, side="right" if i % 2 else "left")
                next(g)  # head
                if prev is not None:
                    for _ in prev:  # tail of previous body, scopes close
                        pass
                prev_fil = next(g)  # pairs + norms
                prev = g
            if prev is not None:
                for _ in prev:
                    pass

        if loop_reps is None:
            run_bodies(_N_BODIES)
        else:
            # benchmark variant: repeat on-device; two pipelined bodies per
            # loop iteration (step=2 keeps the total rep count intact) so
            # consecutive bodies get alternating tile allocations and the
            # next body's projections aren't queued behind the previous
            # body's out-projection.
            ET = mybir.EngineType
            with tc.For_i(
                0,
                loop_reps,
                2,
                hint_engines=(ET.PE, ET.Activation, ET.DVE, ET.SP, ET.Pool),
            ):
                run_bodies(2)

    nc.compile()
    return nc


def get_nc(loop_reps=None):
    global _KT
    if _KT is None:
        _KT = RT  # no inputs seen yet: lock the worst case
    key = ("nc", loop_reps, _KT)
    if key not in _CACHED:
        _CACHED[key] = _build_nc(loop_reps, _KT)
    return _CACHED[key]


def make_in_maps(query, key, value, mask, Wq, bq, Wk, bk, Wv, bv, Wo, bo):
    """Shard full inputs into per-core input maps (host-side numpy)."""
    global _KT
    f = np.float32
    query = np.asarray(query, f)
    key = np.asarray(key, f)
    value = np.asarray(value, f)
    mask = np.asarray(mask)

    import ml_dtypes

    bf = ml_dtypes.bfloat16

    # mask compaction: keep only unmasked key positions, padded to 128
    pos = [np.nonzero(mask[c, 0] != 0)[0] for c in range(B)]
    need = max(1, -(-max(len(p) for p in pos) // P))
    if _KT is None:
        _KT = need
    elif need > _KT:
        _KT = need  # stale cached modules can't serve this input
        _CACHED.clear()
    SK = _KT * P

    def wtiles(W, dt=bf):
        return np.ascontiguousarray(np.asarray(W, f).reshape(KI, P, D).astype(dt))

    wq_t, wk_t, wv_t = wtiles(Wq), wtiles(Wk), wtiles(Wv)
    wo_t = wtiles(Wo)
    bq_t = np.ascontiguousarray(np.asarray(bq, f).reshape(KI, P).T)
    bk_t = np.ascontiguousarray(np.asarray(bk, f).reshape(KI, P).T)
    bv_t = np.ascontiguousarray(np.asarray(bv, f).reshape(1, D).astype(bf))
    bo_t = np.ascontiguousarray(np.asarray(bo, f).reshape(1, D))

    in_maps = []
    for c in range(B):
        L = len(pos[c])
        idx = np.zeros(SK, dtype=np.int64)
        idx[:L] = pos[c]
        mb = np.full(SK, f(MASK_NEG), dtype=f)
        mb[:L] = 0.0
        xqT = np.ascontiguousarray(query[c].T.astype(bf)).reshape(KI, P, S)
        xkT = np.ascontiguousarray(key[c][idx].T.astype(bf)).reshape(KI, P, SK)
        xvT = np.ascontiguousarray(value[c][idx].T.astype(bf)).reshape(KI, P, SK)
        mb = np.ascontiguousarray(mb.reshape(_KT, P).T)
        in_maps.append(
            {
                "xqT": xqT,
                "xkT": xkT,
                "xvT": xvT,
                "maskb": mb,
                "wq": wq_t,
                "wk": wk_t,
                "wv": wv_t,
                "wo": wo_t,
                "bq": bq_t,
                "bk": bk_t,
                "bv": bv_t,
                "bo": bo_t,
            }
        )
    return in_maps


def kernel(**inputs):
    from concourse.bass_utils import run_bass_kernel_spmd

    in_maps = make_in_maps(**inputs)  # before get_nc: locks the k-tile count
    nc = get_nc()
    res = run_bass_kernel_spmd(nc, in_maps, core_ids=list(range(B)))
    out = np.stack([res.results[c]["y"].reshape(S, D) for c in range(B)])
    return out.astype(np.float32)
